# revision 22
# baseline (speedup 1.0000x reference)
"""Block-local self-attention (BlockLocalSelfAttention) on 8 TRN2 NeuronCores.

Sharding: the 32 (batch, head) slices are split 4-per-core (pure data/head
parallelism, no collectives). Each slice is t=4096, d=64, block=128: every
128-query block attends to a 3-block local window plus one global token
(key/value 0), and query 0 additionally attends to all 4096 keys.

Device computes ONLY the block-local window attention, unnormalized and
transposed; everything rank-1-ish (the global-token slot, the global query
row, the softmax normalization, the final transpose) is algebra on tiny
host-side tensors and is folded into the unshard step:

  - K-ordered sweep: for each key block bb, one K=64 matmul produces the
    transposed score tile [128 kk x 384 q] covering query blocks bb-1..bb+1
    (Q is host-padded with a zero block on each side so every window is a
    uniform contiguous 384-column slice). K/Q live in parity row-halves of
    SBUF (even key blocks rows 0:63, odd rows 64:127, Q duplicated) so
    consecutive score matmuls hit disjoint PE row strips and LDWEIGHTS
    overlaps the previous matmul. The zero attention mask plus the
    position-0 local masking ride the exp bias (NEG on partition 0 for key
    block 0); nonzero masks fall back to a host reference implementation.
  - exp() on ScalarE straight out of PSUM into SBUF bf16, in the [kk, q]
    layout the PV matmuls consume as the moving operand.
  - PV accumulates ctx TRANSPOSED: ctxT[d, q] += V[kk, d]^T @ P[kk, q].
    The V block is the stationary (65 cols -> cheap LDWEIGHTS) and each key
    block needs a single N=384 matmul (split only at PSUM bank boundaries).
    V carries a ones column so the softmax denominator lands in ctxT row 64.
    PSUM's has_written bit handles sparse first-touch: the first matmul into
    a bank uses start=True, later ones accumulate/overwrite per element.
  - Completed 512-query ctxT banks are evacuated by VectorE to SBUF and
    DMA'd out as [65, 4096] fp32 per slice.

Host post-pass per (n,h): add the global-slot rank-1 update
(+ pg[q] * v0, + pg[q] to the denominator), divide, transpose, and overwrite
row 0 with the full-softmax global query output.
"""

import os
from contextlib import ExitStack

import ml_dtypes
import numpy as np

N_CORES = 8
N, H, T, D = 2, 16, 4096, 64
BLK = 128
NB = T // BLK           # 32 key/query blocks
S = (N * H) // N_CORES  # 4 slices per core
DA = D + 1              # augmented contraction dim (extra mask/ones row)
VA = D + 1              # V augmented with ones column
NEG = -30000.0          # additive mask value; exp() underflows to exactly 0
QP = T + 2 * BLK        # zero-padded query length
GSZ = int(os.environ.get("KGSZ", "3"))   # key blocks per score tile / exp
NGRP = (NB + GSZ - 1) // GSZ
RP_BUFS = int(os.environ.get("KRPBUFS", "2"))
PT_BUFS = int(os.environ.get("KPTBUFS", "3"))
QBPB = 512 // BLK       # query blocks per PSUM bank (4)

_CACHE = {}
LAST_RESULTS = None  # BassKernelResults of the most recent run (for test.py)


def _install_ntff_shim():
    """Register an antenv.axon_hooks NTFF profile hook backed by direct
    ctypes calls into libaxon_pjrt.so, so trace=True yields a real
    neuron-profile capture in this container. No-op if unavailable."""
    import contextlib
    import ctypes
    import sys
    import types

    if "antenv.axon_hooks" in sys.modules:
        return True
    try:
        lib = ctypes.CDLL("/opt/axon/libaxon_pjrt.so")
        lib.axon_start_nrt_profile.argtypes = [
            ctypes.POINTER(ctypes.c_int64),
            ctypes.c_size_t,
        ]
        lib.axon_start_nrt_profile.restype = ctypes.c_int64
        lib.axon_stop_nrt_profile.argtypes = [ctypes.c_char_p]
        lib.axon_stop_nrt_profile.restype = ctypes.c_int64
    except Exception:
        return False

    @contextlib.contextmanager
    def _hook(output_dir, device_ids):
        import jax

        jax.devices()
        if device_ids:
            ids = (ctypes.c_int64 * len(device_ids))(*device_ids)
            rc = lib.axon_start_nrt_profile(ids, len(device_ids))
        else:
            rc = lib.axon_start_nrt_profile(None, 0)
        if rc != 0:
            raise RuntimeError(f"axon_start_nrt_profile rc={rc}")
        try:
            yield
        finally:
            lib.axon_stop_nrt_profile(str(output_dir).encode())

    mod = types.ModuleType("antenv.axon_hooks")
    mod.get_axon_ntff_profile_hook = lambda: _hook
    mod.set_axon_ntff_profile_hook = lambda h: None
    sys.modules["antenv.axon_hooks"] = mod

    from concourse import bass_utils

    bass_utils.upload_artifacts = lambda tmpdir: f"local:{tmpdir}"
    return True


def _build_program(reps=1):
    import concourse.bass as bass  # noqa: F401
    import concourse.tile as tile
    from concourse import bacc, mybir

    f32 = mybir.dt.float32
    bf16 = mybir.dt.bfloat16
    EXP = mybir.ActivationFunctionType.Exp

    nc = bacc.Bacc("TRN2", target_bir_lowering=False, debug=False)

    # Inputs are pre-chunked on host so each chunk is an independent tile:
    # compute on the first half starts while the second half is in flight.
    # qtp chunks overlap by 2 blocks so no score window straddles them.
    #
    # K/Q use a K=64 contraction (the zero attention_mask rides the exp bias
    # instead of an extra matmul row) and are packed in PARITY row-halves:
    # even key blocks live on SBUF partitions 0..63, odd on 64..127, with Q
    # duplicated in both halves. Consecutive score matmuls then target
    # disjoint PE row strips (tile_position rows 0 / 64), letting the PE
    # pull each LDWEIGHTS ahead of the in-flight previous matmul.
    HKB = NB // 2                 # key blocks per chunk (16)
    QCA = HKB * BLK + 3 * BLK     # qtp chunk A cols (kbs 0..15 windows)
    QCB = QP - HKB * BLK          # qtp chunk B cols (kbs 16..31 windows)
    KC = HKB * BLK // 2           # kt chunk cols (2 kbs per 128-col slot)
    qta_d = nc.dram_tensor("qta", [S, BLK, QCA], bf16, kind="ExternalInput").ap()
    qtb_d = nc.dram_tensor("qtb", [S, BLK, QCB], bf16, kind="ExternalInput").ap()
    kta_d = nc.dram_tensor("kta", [S, BLK, KC], bf16, kind="ExternalInput").ap()
    ktb_d = nc.dram_tensor("ktb", [S, BLK, KC], bf16, kind="ExternalInput").ap()
    va_d = nc.dram_tensor("va", [S, BLK, HKB, VA], bf16, kind="ExternalInput").ap()
    vb_d = nc.dram_tensor("vb", [S, BLK, HKB, VA], bf16, kind="ExternalInput").ap()
    nb_d = nc.dram_tensor("nb", [BLK, 1], f32, kind="ExternalInput").ap()
    out_d = nc.dram_tensor("out", [S, VA, T], bf16, kind="ExternalOutput").ap()

    with tile.TileContext(nc) as tc, ExitStack() as ctx:
        io = ctx.enter_context(tc.tile_pool(name="io", bufs=2))
        rp = ctx.enter_context(tc.tile_pool(name="rp", bufs=RP_BUFS, space="PSUM"))
        cxp = ctx.enter_context(tc.tile_pool(name="cxp", bufs=2, space="PSUM"))
        ptp = ctx.enter_context(tc.tile_pool(name="ptp", bufs=PT_BUFS))
        stp = ctx.enter_context(tc.tile_pool(name="stp", bufs=3))

        # bias column for key block 0's exp: NEG on partition 0 masks the
        # local copy of key position 0 (reachable only via the global slot).
        # Issued on the scalar queue so it doesn't delay slice 0's q/k loads.
        nbt = io.tile([BLK, 1], f32, tag="nb", bufs=1)
        nc.scalar.dma_start(out=nbt, in_=nb_d)

        def build_slice(s):
            # Slice 0 gates kernel start: split its first-needed inputs
            # across both HWDGE queues (sync + scalar; ScalarE is idle until
            # the first exp) so compute starts as early as possible. Later
            # slices prefetch during the previous slice, so latency is
            # hidden and the scalar queue is left alone (ScalarE is the
            # steady-state bottleneck).
            qta = io.tile([BLK, QCA], bf16, tag="qta", bufs=2)
            kta = io.tile([BLK, KC], bf16, tag="kta", bufs=2)
            if s == 0:
                half = (QCA // 2) // BLK * BLK
                nc.sync.dma_start(out=qta[:, 0:half], in_=qta_d[s, :, 0:half])
                nc.scalar.dma_start(out=kta, in_=kta_d[s])
                nc.sync.dma_start(out=qta[:, half:], in_=qta_d[s, :, half:])
            else:
                nc.sync.dma_start(out=qta, in_=qta_d[s])
                nc.sync.dma_start(out=kta, in_=kta_d[s])
            vta = io.tile([BLK, HKB, VA], bf16, tag="va", bufs=2)
            nc.gpsimd.dma_start(out=vta, in_=va_d[s])
            ktb = io.tile([BLK, KC], bf16, tag="ktb", bufs=2)
            nc.sync.dma_start(out=ktb, in_=ktb_d[s])
            qtb = io.tile([BLK, QCB], bf16, tag="qtb", bufs=2)
            nc.gpsimd.dma_start(out=qtb, in_=qtb_d[s])
            vtb = io.tile([BLK, HKB, VA], bf16, tag="vb", bufs=2)
            nc.gpsimd.dma_start(out=vtb, in_=vb_d[s])

            def kt_block(bb):
                # K^T of key block bb on row-half (bb%2): [64, 128]
                t = kta if bb < HKB else ktb
                par, slot = bb % 2, (bb % HKB) // 2
                return t[64 * par : 64 * par + 64, slot * BLK : (slot + 1) * BLK]

            def q_window(bb):
                # padded-q window for key block bb (cols bb*128 .. +384) on
                # the same row-half as kt_block(bb)
                par = bb % 2
                if bb < HKB:
                    return qta[64 * par : 64 * par + 64, bb * BLK : bb * BLK + 3 * BLK]
                o = (bb - HKB) * BLK
                return qtb[64 * par : 64 * par + 64, o : o + 3 * BLK]

            def v_block(bb):
                t = vta if bb < HKB else vtb
                return t[:, bb % HKB, :]

            pts = {}        # group -> SBUF prob tile [128, GSZ, 3*BLK]
            ctx_tiles = {}  # psum bank j -> PSUM ctxT tile [VA, 512]

            def emit_scores(g):
                kbs = range(g * GSZ, min((g + 1) * GSZ, NB))
                r_t = rp.tile([BLK, GSZ, 512], f32, tag="r", bufs=RP_BUFS)
                for i, bb in enumerate(kbs):
                    # scoresT[kk, q] for q window (bb-1..bb+1) via padded Q
                    nc.tensor.matmul(
                        out=r_t[:, i, 0 : 3 * BLK],
                        lhsT=kt_block(bb),
                        rhs=q_window(bb),
                        start=True,
                        stop=True,
                        skip_group_check=True,
                    )
                gsz = len(kbs)
                pt_t = ptp.tile([BLK, GSZ, 3 * BLK], bf16, tag="pt", bufs=PT_BUFS)
                if g == 0:
                    # key block 0: bias NEG on partition 0 masks key pos 0
                    nc.scalar.activation(
                        out=pt_t[:, 0:1, :], in_=r_t[:, 0:1, 0 : 3 * BLK],
                        func=EXP, bias=nbt[:, :],
                    )
                    nc.scalar.activation(
                        out=pt_t[:, 1:gsz, :], in_=r_t[:, 1:gsz, 0 : 3 * BLK],
                        func=EXP,
                    )
                else:
                    nc.scalar.activation(
                        out=pt_t[:, 0:gsz, :], in_=r_t[:, 0:gsz, 0 : 3 * BLK],
                        func=EXP,
                    )
                pts[g] = pt_t

            def emit_pv(g):
                for bb in range(g * GSZ, min((g + 1) * GSZ, NB)):
                    lo, hi = max(bb - 1, 0), min(bb + 1, NB - 1)
                    pt_t = pts[bb // GSZ]
                    for j in range(lo // QBPB, hi // QBPB + 1):
                        b0 = max(lo, QBPB * j)
                        b1 = min(hi, QBPB * j + QBPB - 1)
                        if j not in ctx_tiles:
                            ctx_tiles[j] = cxp.tile(
                                [BLK, 512], f32, tag="ctx", bufs=2,
                                name=f"ctxT_{s}_{j}",
                            )
                        nc.tensor.matmul(
                            out=ctx_tiles[j][
                                0:VA, (b0 - QBPB * j) * BLK : (b1 - QBPB * j + 1) * BLK
                            ],
                            lhsT=v_block(bb),
                            rhs=pt_t[
                                :, bb % GSZ, (b0 - bb + 1) * BLK : (b1 - bb + 2) * BLK
                            ],
                            start=(bb == max(QBPB * j - 1, 0)),
                            stop=(bb == min(QBPB * j + QBPB, NB - 1)),
                            skip_group_check=True,
                        )

            def emit_flush(j):
                stg = stp.tile([VA, 512], bf16, tag="stg", bufs=3)
                nc.vector.tensor_copy(out=stg, in_=ctx_tiles[j][0:VA, :])
                # alternate queues so back-to-back flushes (esp. the final
                # banks of the last slice) drain in parallel
                eng = nc.gpsimd if j % 2 == 0 else nc.sync
                eng.dma_start(out=out_d[s, :, j * 512 : (j + 1) * 512], in_=stg)

            # bank j receives its last PV contribution from key block
            # min(4j+4, 31); flush it right after that block's PV group.
            done_after = {}
            for j in range(NB // QBPB):
                done_after.setdefault(
                    min(QBPB * j + QBPB, NB - 1) // GSZ, []
                ).append(j)

            emit_scores(0)
            for g in range(1, NGRP):
                emit_scores(g)
                emit_pv(g - 1)
                for j in done_after.get(g - 1, ()):
                    emit_flush(j)
            emit_pv(NGRP - 1)
            for j in done_after.get(NGRP - 1, ()):
                emit_flush(j)

        def build_body():
            for s in range(S):
                build_slice(s)

        if reps > 1:
            with tc.For_i(0, reps, 1):
                build_body()
        else:
            build_body()

    nc.compile()
    return nc


def _prep_core_inputs(q, k, v, core):
    bf = ml_dtypes.bfloat16
    scale = np.float32(1.0 / np.sqrt(D))
    HKB = NB // 2
    QCA = HKB * BLK + 3 * BLK
    qtp = np.zeros((S, BLK, QP), np.float32)
    kt2 = np.empty((S, BLK, T // 2), np.float32)
    vt = np.empty((S, BLK, NB, VA), np.float32)
    for s in range(S):
        g = core * S + s
        n, h = divmod(g, H)
        Q, K, V = q[n, h], k[n, h], v[n, h]          # [T, D]
        qs = Q.T * scale                             # [D, T]
        qtp[s, 0:D, BLK : BLK + T] = qs
        qtp[s, 64 : 64 + D, BLK : BLK + T] = qs      # duplicate for row-half 1
        ktp = K.T                                    # [D, T]
        # parity packing: even key block on rows 0:64, odd on rows 64:128
        kb = ktp.reshape(D, NB, BLK)
        kt2[s, 0:D] = kb[:, 0::2].reshape(D, T // 2)
        kt2[s, 64 : 64 + D] = kb[:, 1::2].reshape(D, T // 2)
        va = np.concatenate([V, np.ones((T, 1), np.float32)], axis=1)
        vt[s] = va.reshape(NB, BLK, VA).transpose(1, 0, 2)
    nb = np.zeros((BLK, 1), np.float32)
    nb[0] = NEG
    KC = HKB * BLK // 2
    return {
        "qta": np.ascontiguousarray(qtp[:, :, :QCA]).astype(bf),
        "qtb": np.ascontiguousarray(qtp[:, :, HKB * BLK :]).astype(bf),
        "kta": np.ascontiguousarray(kt2[:, :, :KC]).astype(bf),
        "ktb": np.ascontiguousarray(kt2[:, :, KC:]).astype(bf),
        "va": np.ascontiguousarray(vt[:, :, :HKB]).astype(bf),
        "vb": np.ascontiguousarray(vt[:, :, HKB:]).astype(bf),
        "nb": nb,
    }


def _host_reference(q, k, v, mask):
    """Pure-numpy port of the reference, used only if attention_mask is
    nonzero (the device fast path folds a zero mask into the exp bias)."""
    n, h, t, d = q.shape
    nb = t // BLK
    scale = np.float32(1.0 / np.sqrt(d))
    out = np.empty((n, h, t, d), np.float32)
    idx = np.arange(nb)[:, None] * BLK + np.arange(3 * BLK)[None, :]
    for ni in range(n):
        m = np.asarray(mask[ni, 0, 0], np.float32)
        ml = m.copy()
        ml[0] = np.finfo(np.float32).min
        mlp = np.full(t + 2 * BLK, np.finfo(np.float32).min, np.float32)
        mlp[BLK : BLK + t] = ml
        mb = np.concatenate([np.zeros((nb, 1), np.float32), mlp[idx]], axis=1)
        for hi in range(h):
            Q, K, V = q[ni, hi], k[ni, hi], v[ni, hi]
            kp = np.zeros((t + 2 * BLK, d), np.float32)
            kp[BLK : BLK + t] = K
            vp = np.zeros((t + 2 * BLK, d), np.float32)
            vp[BLK : BLK + t] = V
            kb = np.concatenate([np.broadcast_to(K[0], (nb, 1, d)), kp[idx]], 1)
            vb = np.concatenate([np.broadcast_to(V[0], (nb, 1, d)), vp[idx]], 1)
            qb = Q.reshape(nb, BLK, d)
            sc = np.einsum("nqd,nkd->nqk", qb, kb) * scale + mb[:, None, :]
            sc -= sc.max(-1, keepdims=True)
            p = np.exp(sc)
            p /= p.sum(-1, keepdims=True)
            out[ni, hi] = np.einsum("nqk,nkd->nqd", p, vb).reshape(t, d)
            sg = Q[0] @ K.T * scale + m
            sg -= sg.max()
            pg = np.exp(sg)
            out[ni, hi, 0] = (pg / pg.sum()) @ V
    return out


def kernel(query_layer, key_layer, value_layer, attention_mask):
    global LAST_RESULTS
    from concourse.bass_utils import run_bass_kernel_spmd

    q = np.ascontiguousarray(np.asarray(query_layer, dtype=np.float32))
    k = np.ascontiguousarray(np.asarray(key_layer, dtype=np.float32))
    v = np.ascontiguousarray(np.asarray(value_layer, dtype=np.float32))
    mask = np.asarray(attention_mask, dtype=np.float32)

    if np.abs(mask).max() != 0:
        # device fast path assumes zero mask; stay correct for any input
        return _host_reference(q, k, v, mask)

    if "nc" not in _CACHE:
        _CACHE["nc"] = _build_program()
    nc = _CACHE["nc"]

    in_maps = [_prep_core_inputs(q, k, v, c) for c in range(N_CORES)]
    trace = bool(int(os.environ.get("KERNEL_TRACE", "0")))
    if trace:
        trace = _install_ntff_shim()
    res = run_bass_kernel_spmd(nc, in_maps, list(range(N_CORES)), trace=trace)
    LAST_RESULTS = res

    # ---- host post-pass: global-token slot + normalize + global query ----
    G = N * H
    scale = np.float32(1.0 / np.sqrt(D))
    qf = q.reshape(G, T, D)
    kf = k.reshape(G, T, D)
    vf = v.reshape(G, T, D)
    mf = np.broadcast_to(mask.reshape(N, 1, 1, T), (N, H, 1, T)).reshape(G, T)

    ctxT = np.empty((G, VA, T), np.float32)
    for c in range(N_CORES):
        # [S, VA, T] bf16 on device; upcast on host
        ctxT[c * S : (c + 1) * S] = np.asarray(res.results[c]["out"], np.float32)

    # global-token slot: every query attends key/value 0 with additive mask 0
    pg = np.exp(scale * np.einsum("gtd,gd->gt", qf, kf[:, 0]))     # [G, T]
    numer = ctxT[:, :D, :] + vf[:, 0][:, :, None] * pg[:, None, :]  # [G, D, T]
    denom = ctxT[:, D, :] + pg                                      # [G, T]
    out = np.ascontiguousarray((numer / denom[:, None, :]).transpose(0, 2, 1))

    # global query: row 0 attends ALL keys (full softmax, raw mask)
    s0 = scale * np.einsum("gd,gtd->gt", qf[:, 0], kf) + mf         # [G, T]
    s0 -= s0.max(axis=-1, keepdims=True)
    p0 = np.exp(s0)
    p0 /= p0.sum(axis=-1, keepdims=True)
    out[:, 0, :] = np.einsum("gt,gtd->gd", p0, vf)

    return out.reshape(N, H, T, D)
